# revision 25
# baseline (speedup 1.0000x reference)
"""Trainium2 Bass kernel for single-CLS-query attention.

Reference computation (per batch b):
    q   = (x[b,0,:] @ Wq.T) * d**-0.5                  # (C,)  single CLS query
    k   = x[b] @ Wk.T ; v = x[b] @ Wv.T                # (N,C)
    s   = per-head dot(q, k) + mask                    # (N,H)
    p   = softmax(s, axis=N)
    out = per-head sum_n p[n,h] v[n,h*64:(h+1)*64]     # (C,)
    y   = out @ Wp.T + bp

Key algebraic restructuring (exploits the single query):
    qhat[h,:] = sum_d q[h*64+d] * Wk[h*64+d,:]         # (H,C)  fold q through Wk
    s         = x @ qhat.T                             # skinny matmul, no k!
    z[h,:]    = sum_n p[n,h] * x[b,n,:]                # (H,C)  fold p into x
    out'      = z @ Wv.T  (full 16x1024 cross)         # block-diag extract -> out
This removes both dense projections x@Wk.T / x@Wv.T (~137 GFLOP -> ~2 GFLOP)
and makes the kernel memory-bound on streaming x.

x is streamed twice: once transposed (C on partitions) for the s-matmul, once
natural (N on partitions) for the z accumulation. The transposed copy only
feeds the softmax logits, so it ships as fp8e4m3 (half the bytes; measured
end-to-end rel-err ~9e-3 vs the 2e-2 gate). The natural copy stays bf16.
Both copies are host-reordered so every DMA lands as large fully-contiguous
per-partition descriptors (8-16KB), one dma_start per quarter-batch; the
profiled baseline lost ~90us to per-dma_start sync-engine serialization
(163 issues x ~0.7us) plus repeated HAM clock-throttle from TensorE gaps.

The additive mask is folded into the s-matmul PSUM group as a 9th
accumulation matmul (ones[1,16].T @ mask_row[1,n]), and the softmax
denominator comes free from the Exp activation's accum_out, so the whole
p-production path is: matmuls -> one fused exp -> 8 tiny transposes.

Sharding: data-parallel over batch. 8 cores x 2 batches each. No collectives.
softmax is computed without max-subtraction: logits here are ~N(0, 0.4), far
inside fp32 exp range (mask is additive zeros in this problem's distribution).
"""

import numpy as np
from contextlib import ExitStack

import concourse.bass as bass
from concourse import bacc
import concourse.tile as tile
from concourse import mybir
from concourse import bass_utils
from concourse.masks import make_identity

B, N, C, H, D = 16, 4096, 1024, 16, 64
NCORES = 8
BPC = B // NCORES          # batches per core
SCALE = float(D) ** -0.5
F32 = mybir.dt.float32
BF16 = mybir.dt.bfloat16
FP8 = mybir.dt.float8e4
CB = C // 128              # 8 contraction blocks of 128 channels
Q = 4                      # quarters per batch (DMA granule)
QN = N // Q                # 1024 rows per quarter
E = 8                      # eighths per batch (PSUM/pipeline granule)
EN = N // E                # 512 rows per eighth
RP = EN // 128             # 4 rows per partition within an eighth

AF = mybir.ActivationFunctionType
ALU = mybir.AluOpType


def _bc(ap_slice, parts):
    """Broadcast an AP (leading dim of size 1, or 1-D) over `parts` partitions."""
    dims = [list(p) for p in ap_slice.ap]
    if len(dims) > 1 and dims[0][1] == 1:
        dims = dims[1:]
    return bass.AP(
        tensor=ap_slice.tensor,
        offset=ap_slice.offset,
        ap=[[0, parts]] + dims,
    )


def build_module():
    nc = bacc.Bacc(target_bir_lowering=False, trn_type="TRN2")

    x_d = nc.dram_tensor("xb", [BPC, N, C], BF16, kind="ExternalInput")
    xt8_d = nc.dram_tensor("xt8", [BPC, Q, 128, CB, QN], FP8, kind="ExternalInput")
    qh_d = nc.dram_tensor("qhT", [BPC, 128, CB, 2 * H], BF16, kind="ExternalInput")
    mrow_d = nc.dram_tensor("mrow", [BPC, N], BF16, kind="ExternalInput")
    sum4_d = nc.dram_tensor("sum4", [128, H], BF16, kind="ExternalInput")
    wv_d = nc.dram_tensor("WvT", [128, CB, C], BF16, kind="ExternalInput")
    wp_d = nc.dram_tensor("WpT", [128, CB, C], BF16, kind="ExternalInput")
    bp_d = nc.dram_tensor("bp", [C], F32, kind="ExternalInput")
    y_d = nc.dram_tensor("y", [BPC, C], F32, kind="ExternalOutput")

    with tile.TileContext(nc) as tc, ExitStack() as ctx:
        singles = ctx.enter_context(tc.tile_pool(name="singles", bufs=1))
        perb = ctx.enter_context(tc.tile_pool(name="perb", bufs=2))
        xtq = ctx.enter_context(tc.tile_pool(name="xtq", bufs=5))
        xinq = ctx.enter_context(tc.tile_pool(name="xinq", bufs=5))
        sbw = ctx.enter_context(tc.tile_pool(name="sbw", bufs=3))
        smalls = ctx.enter_context(tc.tile_pool(name="smalls", bufs=12))
        psST = ctx.enter_context(tc.tile_pool(name="psST", bufs=2, space="PSUM"))
        psZP = ctx.enter_context(tc.tile_pool(name="psZP", bufs=1, space="PSUM"))
        psE = ctx.enter_context(tc.tile_pool(name="psE", bufs=2, space="PSUM"))
        psT = ctx.enter_context(tc.tile_pool(name="psT", bufs=2, space="PSUM"))

        ident_bf = singles.tile([128, 128], BF16)
        make_identity(nc, ident_bf)

        ones16 = singles.tile([1, H], BF16)
        nc.vector.memset(ones16, 1.0)

        # per-batch tiny tensors: folded query (C,H) and mask row (1,N)
        qhs, mrows = [None] * BPC, [None] * BPC
        bp_state = {}

        def emit_small(b):
            qh = perb.tile([128, CB, 2 * H], BF16, tag="qh", name=f"qh{b}")
            nc.sync.dma_start(out=qh, in_=qh_d[b])
            qhs[b] = qh
            mrow = perb.tile([1, N], BF16, tag="mrow", name=f"mrow{b}")
            nc.sync.dma_start(out=mrow, in_=mrow_d[b])
            mrows[b] = mrow

        # streamed quarter tiles: transposed fp8 (s input) + natural bf16 (z input)
        qtiles = {}

        xt_tiles, xi_tiles = {}, {}

        # stream DMAs land at eighth granularity: the per-eighth wait when
        # compute catches the (saturated) DMA stays under the ~3.4us HAM
        # window, so the PE clock never drops to 1.2 GHz mid-stream.
        def emit_xt_h(ei):
            qi, h = divmod(ei, 2)
            b, q = divmod(qi, Q)
            if h == 0:
                xt_tiles[qi] = xtq.tile(
                    [128, CB, QN], FP8, tag="xt", name=f"xt{b}_{q}")
            xt = xt_tiles[qi]
            nc.sync.dma_start(
                out=xt[:, :, h * EN:(h + 1) * EN],
                in_=xt8_d[b, q, :, :, h * EN:(h + 1) * EN])

        def emit_xi_h(ei):
            qi, h = divmod(ei, 2)
            b, q = divmod(qi, Q)
            if h == 0:
                xi_tiles[qi] = xinq.tile(
                    [128, 2, RP, C], BF16, tag="xin", name=f"xi{b}_{q}")
            xi = xi_tiles[qi]
            src = x_d[b, q * QN + h * EN: q * QN + (h + 1) * EN, :].rearrange(
                "(p r) c -> p r c", r=RP)
            nc.sync.dma_start(out=xi[:, h, :, :], in_=src)

        wts = {}

        def load_w(nm):
            wt_d = {"v": wv_d, "p": wp_d}[nm]
            w = singles.tile([128, CB, C], BF16, tag=f"w_{nm}", name=f"w_{nm}")
            nc.sync.dma_start(out=w, in_=wt_d[:])
            wts[nm] = w

        NE = BPC * E           # 16 eighths total
        spsb_tiles = {}
        sT_tiles = {}
        pT_tiles = {}
        zp_tiles = {}
        l_parts = {b: [] for b in range(BPC)}
        linvs = {}
        zpsb_tiles = {}
        sum4_state = {}

        def compute_s(ei):
            """s-matmuls for eighth ei straight into one PSUM bank, then exp.
            (A col-tiled variant was tried here: the partials->SBUF->reduce
            hops starved TensorE and lost more to HAM throttling than the
            concurrency won back.)"""
            b, e = divmod(ei, E)
            xt = xt_tiles[ei // 2]
            cols = slice((ei % 2) * EN, (ei % 2 + 1) * EN)
            sT = psST.tile([16, EN], F32, tag="sT", name=f"sT{ei}")
            for k in range(CB):
                nc.tensor.matmul(
                    sT, qhs[b][:, k, 0:H], xt[:, k, cols],
                    start=(k == 0), stop=False,
                )
            nc.tensor.matmul(
                sT, ones16, mrows[b][:, e * EN:(e + 1) * EN],
                start=False, stop=True,
            )
            # fused: PSUM->SBUF move + exp + softmax-denominator partial
            pT = sbw.tile([16, EN], BF16, tag="pT", name=f"pT{ei}", bufs=3)
            lq = smalls.tile([16, 1], F32, tag="lq", name=f"lq{ei}")
            nc.scalar.activation(out=pT, in_=sT, func=AF.Exp, accum_out=lq)
            l_parts[b].append(lq)
            pT_tiles[ei] = pT

        def compute_tz(ei):
            """transpose p to natural layout (n on partitions)."""
            pT = pT_tiles.pop(ei)
            tp = psT.tile([128, RP * 32], F32, tag="tp", name=f"tp{ei}")
            for r in range(RP):
                # plain matmul against a zero-padded identity: cols 16-31 of
                # each 32-group come out as hard zeros (stationary padding for
                # the col-tiled z matmuls)
                nc.tensor.matmul(
                    tp[:, r * 32:(r + 1) * 32], pT[:, r::RP],
                    ident_bf[0:16, 0:32], start=True, stop=True,
                )
            p_nat = sbw.tile([128, RP * 32], BF16, tag="p_nat", name=f"pn{ei}", bufs=2)
            nc.vector.tensor_copy(out=p_nat, in_=tp)
            return p_nat

        def compute_z(ei, p_nat):
            """col-tiled z partials: 4 r-rows concurrent, accumulated over the
            whole batch into 4 PSUM partition groups."""
            b, e = divmod(ei, E)
            xi = xi_tiles[ei // 2]
            if e == E - 1:
                xt_tiles.pop(ei // 2)
                xi_tiles.pop(ei // 2)
            if e == 0:
                zp_tiles[b] = psZP.tile([128, C], F32, tag="zp", name=f"zp{b}")
            zp = zp_tiles[b]
            for r in range(RP):
                for cc in range(2):
                    nc.tensor.matmul(
                        zp[32 * r:32 * r + 32, cc * 512:(cc + 1) * 512],
                        p_nat[:, r * 32:r * 32 + 32],
                        xi[:, ei % 2, r, cc * 512:(cc + 1) * 512],
                        start=(e == 0),
                        stop=(e == E - 1),
                        tile_position=(0, 32 * r),
                        skip_group_check=True,
                    )

        def epilogue_pre(b):
            """softmax denominator + z partials to SBUF (vector); frees zp."""
            zp = zp_tiles.pop(b)
            lp = l_parts[b]
            while len(lp) > 1:
                nxt = []
                for i in range(0, len(lp) - 1, 2):
                    ls = smalls.tile([16, 1], F32, tag="lq", name=f"ls{b}_{len(lp)}_{i}")
                    nc.vector.tensor_tensor(out=ls, in0=lp[i], in1=lp[i + 1], op=ALU.add)
                    nxt.append(ls)
                if len(lp) % 2:
                    nxt.append(lp[-1])
                lp = nxt
            linv = smalls.tile([16, 1], F32, tag="lq", name=f"li_{b}")
            nc.vector.reciprocal(out=linv, in_=lp[0])
            linvs[b] = linv
            zp_sb = sbw.tile([128, C], BF16, tag="zp_sb", name=f"zpsb{b}", bufs=2)
            nc.vector.tensor_copy(out=zp_sb, in_=zp)
            zpsb_tiles[b] = zp_sb

        def epilogue_main(b):
            """tail for batch b: reduce+normalize z, then the two projections."""
            z_sb = sbw.tile([16, C], BF16, tag="z_sb", name=f"zsb{b}", bufs=2)
            for cc in range(2):
                zred = psE.tile([16, 512], F32, tag="acc", name=f"zred{b}_{cc}")
                nc.tensor.matmul(
                    zred, sum4_state["t"],
                    zpsb_tiles[b][:, cc * 512:(cc + 1) * 512],
                    start=True, stop=True,
                )
                nc.vector.tensor_scalar_mul(
                    z_sb[:, cc * 512:(cc + 1) * 512], zred, linvs[b])

            # zT[c_p, k*16+h] for the Wv cross product
            tpz = psT.tile([128, 128], BF16, tag="tp", name=f"tpz{b}")
            for k in range(CB):
                nc.tensor.transpose(
                    tpz[:, k * 16:(k + 1) * 16],
                    z_sb[:, k * 128:(k + 1) * 128],
                    ident_bf[0:16, 0:16],
                )
            zT_sb = sbw.tile([128, 128], BF16, tag="zT", name=f"zT{b}", bufs=2)
            nc.vector.tensor_copy(out=zT_sb, in_=tpz)

            # out' = z @ Wv.T (full HxC cross)
            outp_sb = sbw.tile([16, C], BF16, tag="outp", name=f"osb{b}", bufs=2)
            for cc in range(2):
                outp = psE.tile([16, 512], F32, tag="acc", name=f"outp{b}_{cc}")
                for k in range(CB):
                    nc.tensor.matmul(
                        outp,
                        zT_sb[:, k * 16:(k + 1) * 16],
                        wts["v"][:, k, cc * 512:(cc + 1) * 512],
                        start=(k == 0), stop=(k == CB - 1),
                    )
                nc.vector.tensor_copy(
                    out=outp_sb[:, cc * 512:(cc + 1) * 512], in_=outp)

            # block-diagonal extract: out[j*128+row] lives at head 2j+(row>=64)
            tpo = psT.tile([128, 128], BF16, tag="tp", name=f"tpo{b}")
            for j in range(CB):
                nc.tensor.transpose(
                    tpo[:, j * 16:(j + 1) * 16],
                    outp_sb[:, j * 128:(j + 1) * 128],
                    ident_bf[0:16, 0:16],
                )
            oc_sb = sbw.tile([128, CB], BF16, tag="oc", name=f"oc{b}", bufs=2)
            nc.vector.tensor_copy(out=oc_sb[0:64, :], in_=tpo[0:64, 0::18])
            nc.vector.tensor_copy(out=oc_sb[64:128, :], in_=tpo[64:128, 1::18])

            # y = out @ Wp.T + bp
            y_sb = sbw.tile([1, C], F32, tag="y", name=f"y{b}", bufs=2)
            for cc in range(2):
                y_ps = psE.tile([1, 512], F32, tag="acc", name=f"yps{b}_{cc}")
                for j in range(CB):
                    nc.tensor.matmul(
                        y_ps,
                        oc_sb[:, j:j + 1],
                        wts["p"][:, j, cc * 512:(cc + 1) * 512],
                        start=(j == 0), stop=(j == CB - 1),
                    )
                nc.vector.tensor_tensor(
                    out=y_sb[:, cc * 512:(cc + 1) * 512], in0=y_ps,
                    in1=bp_state["bp"][0:1, cc * 512:(cc + 1) * 512], op=ALU.add)
            nc.sync.dma_start(out=y_d[b, :], in_=y_sb)

        # ---- schedule: two-eighth software pipeline skew; xt leads xi in
        # the DMA FIFO, weights slot in mid-stream (needed first by epi(0)).
        qh = perb.tile([128, CB, 2 * H], BF16, tag="qh", name="qh0")
        nc.sync.dma_start(out=qh, in_=qh_d[0])
        qhs[0] = qh
        sum4 = singles.tile([128, H], BF16, name="sum4")
        nc.sync.dma_start(out=sum4, in_=sum4_d[:])
        sum4_state["t"] = sum4
        emit_xt_h(0)
        mrow = perb.tile([1, N], BF16, tag="mrow", name="mrow0")
        nc.sync.dma_start(out=mrow, in_=mrow_d[0])
        mrows[0] = mrow
        emit_xt_h(1)
        emit_xi_h(0)
        emit_xt_h(2)
        emit_xi_h(1)
        bp_row = singles.tile([2, C], F32, name="bp_row")
        nc.sync.dma_start(out=bp_row, in_=_bc(bp_d[:], BPC))
        bp_state["bp"] = bp_row
        emit_small(1)
        emit_xt_h(3)

        compute_s(0)
        compute_s(1)
        for ei in range(NE):
            if ei + 4 < NE:
                emit_xt_h(ei + 4)
            if ei + 2 < NE:
                emit_xi_h(ei + 2)
            if ei == 2:
                load_w("v")
            elif ei == 4:
                load_w("p")
            p_nat = compute_tz(ei)
            if ei + 2 < NE:
                compute_s(ei + 2)
            compute_z(ei, p_nat)
            if ei % E == E - 1:
                epilogue_pre(ei // E)
            if ei == E:
                epilogue_main(0)
        epilogue_main(1)

    nc.compile()
    return nc


def _ensure_ntff_hook():
    """The agent image's antenv lacks axon_hooks; synthesize it and install
    the ctypes NTFF profile hook from trn_boot so trace=True works."""
    import sys
    import types
    try:
        from antenv.axon_hooks import get_axon_ntff_profile_hook  # noqa: F401
        return
    except ImportError:
        pass
    import antenv
    mod = types.ModuleType("antenv.axon_hooks")
    state = {}
    mod.set_axon_ntff_profile_hook = lambda h: state.__setitem__("h", h)
    mod.get_axon_ntff_profile_hook = lambda: state.get("h")
    sys.modules["antenv.axon_hooks"] = mod
    antenv.axon_hooks = mod
    try:
        from trn_agent_boot.trn_boot import _ntff_profile_via_ctypes
        mod.set_axon_ntff_profile_hook(
            _ntff_profile_via_ctypes("/opt/axon/libaxon_pjrt.so")
        )
    except Exception:
        pass


_NC_CACHE = None


def _get_module():
    global _NC_CACHE
    if _NC_CACHE is None:
        _NC_CACHE = build_module()
    return _NC_CACHE


def _prep_inputs(inputs):
    """Host-side prep: bf16/fp8 casts, DMA-friendly reorders, per-batch qhat."""
    import ml_dtypes
    bf16 = ml_dtypes.bfloat16
    f8 = ml_dtypes.float8_e4m3

    x = np.ascontiguousarray(inputs["x"], dtype=np.float32)       # (B,N,C)
    mask = np.ascontiguousarray(inputs["mask"], dtype=np.float32)
    Wq = np.asarray(inputs["Wq"], dtype=np.float32)
    Wk = np.asarray(inputs["Wk"], dtype=np.float32)

    xb = x.astype(bf16)                                            # (B,N,C)
    # transposed copy in fp8, reordered to [B, Q, 128, CB, QN]:
    # (b,q,p,k,n') = x[b, q*QN+n', k*128+p]
    xt = x.transpose(0, 2, 1)                                      # (B,C,N)
    xt8 = np.ascontiguousarray(
        xt.reshape(B, CB, 128, Q, QN).transpose(0, 3, 2, 1, 4)
    ).astype(f8)

    # qhat[b,h,:] = sum_d (x[b,0] @ Wq.T * scale)[h*64+d] * Wk[h*64+d,:]
    q = (x[:, 0, :].astype(np.float64) @ Wq.T.astype(np.float64)) * SCALE  # (B,C)
    qhd = q.reshape(B, H, D)
    Wkh = Wk.reshape(H, D, C).astype(np.float64)
    qhat = np.einsum("bhd,hdc->bhc", qhd, Wkh)                     # (B,H,C)
    # [B, 128, CB, 2H]: (b,p,k,h) = qhat[b, h, k*128+p], heads 16-31 zero-padded
    qhT = np.ascontiguousarray(
        qhat.transpose(0, 2, 1).reshape(B, CB, 128, H).transpose(0, 2, 1, 3))
    qhT = np.concatenate([qhT, np.zeros_like(qhT)], axis=3).astype(bf16)

    # mask_full = [0, mask[b]] as a single bf16 row per batch
    mrow = np.concatenate(
        [np.zeros((B, 1), np.float32), mask], axis=1).astype(bf16)  # (B,N)

    def reorder_w(w):  # (C,C) -> [128, CB, C] with (p,k,c) = W[c, k*128+p]
        wt = np.ascontiguousarray(np.asarray(w, np.float32).T)      # (C,C) W.T
        return np.ascontiguousarray(
            wt.reshape(CB, 128, C).transpose(1, 0, 2)).astype(bf16)

    # reduction stationary for the 4 col-tiled partition groups:
    # sum4[32g+h, h] = 1  (g = array col-group, h = head)
    sum4 = np.zeros((128, H), dtype=np.float32)
    for g in range(4):
        sum4[32 * g:32 * g + H, :] = np.eye(H)
    shared = {
        "WvT": reorder_w(inputs["Wv"]),
        "WpT": reorder_w(inputs["Wp"]),
        "bp": np.ascontiguousarray(inputs["bp"], dtype=np.float32),
        "sum4": sum4.astype(bf16),
    }
    in_maps = []
    for c in range(NCORES):
        sl = slice(c * BPC, (c + 1) * BPC)
        m = {
            "xb": xb[sl], "xt8": xt8[sl], "qhT": qhT[sl],
            "mrow": mrow[sl],
        }
        m.update(shared)
        in_maps.append(m)
    return in_maps


def run(inputs, trace=False):
    if trace:
        _ensure_ntff_hook()
    nc = _get_module()
    in_maps = _prep_inputs(inputs)
    res = bass_utils.run_bass_kernel_spmd(
        nc, in_maps, core_ids=list(range(NCORES)), trace=trace
    )
    ys = [res.results[c]["y"] for c in range(NCORES)]
    out = np.concatenate(ys, axis=0).reshape(B, 1, C)
    return out, res


def kernel(**inputs):
    out, _ = run(inputs, trace=False)
    return out


if __name__ == "__main__":
    rng = np.random.default_rng(0)
    ins = {
        "x": rng.standard_normal((B, N, C), dtype=np.float32),
        "mask": np.zeros((B, N - 1), dtype=np.float32),
        "Wq": (rng.standard_normal((C, C)) * 0.02).astype(np.float32),
        "Wk": (rng.standard_normal((C, C)) * 0.02).astype(np.float32),
        "Wv": (rng.standard_normal((C, C)) * 0.02).astype(np.float32),
        "Wp": (rng.standard_normal((C, C)) * 0.02).astype(np.float32),
        "bp": np.zeros((C,), dtype=np.float32),
    }
    y = kernel(**ins)
    print(y.shape, y.dtype, np.abs(y).mean())


# revision 26
# speedup vs baseline: 1.1618x; 1.1618x over previous
"""Trainium2 Bass kernel for single-CLS-query attention.

Reference computation (per batch b):
    q   = (x[b,0,:] @ Wq.T) * d**-0.5                  # (C,)  single CLS query
    k   = x[b] @ Wk.T ; v = x[b] @ Wv.T                # (N,C)
    s   = per-head dot(q, k) + mask                    # (N,H)
    p   = softmax(s, axis=N)
    out = per-head sum_n p[n,h] v[n,h*64:(h+1)*64]     # (C,)
    y   = out @ Wp.T + bp

Key algebraic restructuring (exploits the single query):
    qhat[h,:] = sum_d q[h*64+d] * Wk[h*64+d,:]         # (H,C)  fold q through Wk
    s         = x @ qhat.T                             # skinny matmul, no k!
    z[h,:]    = sum_n p[n,h] * x[b,n,:]                # (H,C)  fold p into x
    out'      = z @ Wv.T  (full 16x1024 cross)         # block-diag extract -> out
This removes both dense projections x@Wk.T / x@Wv.T (~137 GFLOP -> ~2 GFLOP)
and makes the kernel memory-bound on streaming x.

x is streamed twice: once transposed (C on partitions) for the s-matmul, once
natural (N on partitions) for the z accumulation. The transposed copy only
feeds the softmax logits, so it ships as fp8e4m3 (half the bytes; measured
end-to-end rel-err ~9e-3 vs the 2e-2 gate). The natural copy stays bf16.
Both copies are host-reordered so every DMA lands as large fully-contiguous
per-partition descriptors (8-16KB), one dma_start per quarter-batch; the
profiled baseline lost ~90us to per-dma_start sync-engine serialization
(163 issues x ~0.7us) plus repeated HAM clock-throttle from TensorE gaps.

The additive mask is folded into the s-matmul PSUM group as a 9th
accumulation matmul (ones[1,16].T @ mask_row[1,n]), and the softmax
denominator comes free from the Exp activation's accum_out, so the whole
p-production path is: matmuls -> one fused exp -> 8 tiny transposes.

Sharding: data-parallel over batch. 8 cores x 2 batches each. No collectives.
softmax is computed without max-subtraction: logits here are ~N(0, 0.4), far
inside fp32 exp range (mask is additive zeros in this problem's distribution).
"""

import numpy as np
from contextlib import ExitStack

import concourse.bass as bass
from concourse import bacc
import concourse.tile as tile
from concourse import mybir
from concourse import bass_utils
from concourse.masks import make_identity

B, N, C, H, D = 16, 4096, 1024, 16, 64
NCORES = 8
BPC = B // NCORES          # batches per core
SCALE = float(D) ** -0.5
F32 = mybir.dt.float32
BF16 = mybir.dt.bfloat16
FP8 = mybir.dt.float8e4
CB = C // 128              # 8 contraction blocks of 128 channels
Q = 4                      # quarters per batch (DMA granule)
QN = N // Q                # 1024 rows per quarter
E = 8                      # eighths per batch (PSUM/pipeline granule)
EN = N // E                # 512 rows per eighth
RP = EN // 128             # 4 rows per partition within an eighth

AF = mybir.ActivationFunctionType
ALU = mybir.AluOpType


def _bc(ap_slice, parts):
    """Broadcast an AP (leading dim of size 1, or 1-D) over `parts` partitions."""
    dims = [list(p) for p in ap_slice.ap]
    if len(dims) > 1 and dims[0][1] == 1:
        dims = dims[1:]
    return bass.AP(
        tensor=ap_slice.tensor,
        offset=ap_slice.offset,
        ap=[[0, parts]] + dims,
    )


def build_module():
    nc = bacc.Bacc(target_bir_lowering=False, trn_type="TRN2")

    x_d = nc.dram_tensor("xb", [BPC, N, C], BF16, kind="ExternalInput")
    xt8_d = nc.dram_tensor("xt8", [BPC, Q, 128, CB, QN], FP8, kind="ExternalInput")
    qh_d = nc.dram_tensor("qhT", [BPC, 128, CB, 2 * H], BF16, kind="ExternalInput")
    mrow_d = nc.dram_tensor("mrow", [BPC, N], BF16, kind="ExternalInput")
    sum4_d = nc.dram_tensor("sum4", [128, H], BF16, kind="ExternalInput")
    wv_d = nc.dram_tensor("WvT", [128, CB, C], BF16, kind="ExternalInput")
    wp_d = nc.dram_tensor("WpT", [128, CB, C], BF16, kind="ExternalInput")
    bp_d = nc.dram_tensor("bp", [C], F32, kind="ExternalInput")
    y_d = nc.dram_tensor("y", [BPC, C], F32, kind="ExternalOutput")

    with tile.TileContext(nc) as tc, ExitStack() as ctx:
        singles = ctx.enter_context(tc.tile_pool(name="singles", bufs=1))
        perb = ctx.enter_context(tc.tile_pool(name="perb", bufs=2))
        xtq = ctx.enter_context(tc.tile_pool(name="xtq", bufs=5))
        xinq = ctx.enter_context(tc.tile_pool(name="xinq", bufs=5))
        sbw = ctx.enter_context(tc.tile_pool(name="sbw", bufs=3))
        smalls = ctx.enter_context(tc.tile_pool(name="smalls", bufs=12))
        psST = ctx.enter_context(tc.tile_pool(name="psST", bufs=2, space="PSUM"))
        psZP = ctx.enter_context(tc.tile_pool(name="psZP", bufs=1, space="PSUM"))
        psE = ctx.enter_context(tc.tile_pool(name="psE", bufs=2, space="PSUM"))
        psT = ctx.enter_context(tc.tile_pool(name="psT", bufs=2, space="PSUM"))

        ident_bf = singles.tile([128, 128], BF16)
        make_identity(nc, ident_bf)

        ones16 = singles.tile([1, H], BF16)
        nc.vector.memset(ones16, 1.0)

        # per-batch tiny tensors: folded query (C,H) and mask row (1,N)
        qhs, mrows = [None] * BPC, [None] * BPC
        bp_state = {}

        def emit_small(b):
            qh = perb.tile([128, CB, 2 * H], BF16, tag="qh", name=f"qh{b}")
            nc.sync.dma_start(out=qh, in_=qh_d[b])
            qhs[b] = qh
            mrow = perb.tile([1, N], BF16, tag="mrow", name=f"mrow{b}")
            nc.sync.dma_start(out=mrow, in_=mrow_d[b])
            mrows[b] = mrow

        # streamed quarter tiles: transposed fp8 (s input) + natural bf16 (z input)
        qtiles = {}

        xt_tiles, xi_tiles = {}, {}

        # stream DMAs land at eighth granularity: the per-eighth wait when
        # compute catches the (saturated) DMA stays under the ~3.4us HAM
        # window, so the PE clock never drops to 1.2 GHz mid-stream.
        def emit_xt_h(ei):
            # one DMA per quarter (1KB descriptors; a per-eighth split was
            # measured slower: 512B descriptors drop DMA efficiency ~15%)
            qi, h = divmod(ei, 2)
            if h != 0:
                return
            b, q = divmod(qi, Q)
            xt_tiles[qi] = xtq.tile(
                [128, CB, QN], FP8, tag="xt", name=f"xt{b}_{q}")
            nc.sync.dma_start(out=xt_tiles[qi], in_=xt8_d[b, q])

        def emit_xi_h(ei):
            qi, h = divmod(ei, 2)
            b, q = divmod(qi, Q)
            if h == 0:
                xi_tiles[qi] = xinq.tile(
                    [128, 2, RP, C], BF16, tag="xin", name=f"xi{b}_{q}")
            xi = xi_tiles[qi]
            src = x_d[b, q * QN + h * EN: q * QN + (h + 1) * EN, :].rearrange(
                "(p r) c -> p r c", r=RP)
            nc.sync.dma_start(out=xi[:, h, :, :], in_=src)

        wts = {}

        def load_w(nm):
            wt_d = {"v": wv_d, "p": wp_d}[nm]
            w = singles.tile([128, CB, C], BF16, tag=f"w_{nm}", name=f"w_{nm}")
            nc.sync.dma_start(out=w, in_=wt_d[:])
            wts[nm] = w

        NE = BPC * E           # 16 eighths total
        spsb_tiles = {}
        sT_tiles = {}
        pT_tiles = {}
        zp_tiles = {}
        l_parts = {b: [] for b in range(BPC)}
        linvs = {}
        zpsb_tiles = {}
        sum4_state = {}

        def compute_s(ei):
            """s-matmuls for eighth ei straight into one PSUM bank, then exp.
            (A col-tiled variant was tried here: the partials->SBUF->reduce
            hops starved TensorE and lost more to HAM throttling than the
            concurrency won back.)"""
            b, e = divmod(ei, E)
            xt = xt_tiles[ei // 2]
            cols = slice((ei % 2) * EN, (ei % 2 + 1) * EN)
            sT = psST.tile([16, EN], F32, tag="sT", name=f"sT{ei}")
            for k in range(CB):
                nc.tensor.matmul(
                    sT, qhs[b][:, k, 0:H], xt[:, k, cols],
                    start=(k == 0), stop=False,
                )
            nc.tensor.matmul(
                sT, ones16, mrows[b][:, e * EN:(e + 1) * EN],
                start=False, stop=True,
            )
            # fused: PSUM->SBUF move + exp + softmax-denominator partial
            pT = sbw.tile([16, EN], BF16, tag="pT", name=f"pT{ei}", bufs=3)
            lq = smalls.tile([16, 1], F32, tag="lq", name=f"lq{ei}")
            nc.scalar.activation(out=pT, in_=sT, func=AF.Exp, accum_out=lq)
            l_parts[b].append(lq)
            pT_tiles[ei] = pT

        def compute_tz(ei):
            """transpose p to natural layout (n on partitions)."""
            pT = pT_tiles.pop(ei)
            tp = psT.tile([128, RP * 32], F32, tag="tp", name=f"tp{ei}")
            for r in range(RP):
                # plain matmul against a zero-padded identity: cols 16-31 of
                # each 32-group come out as hard zeros (stationary padding for
                # the col-tiled z matmuls)
                nc.tensor.matmul(
                    tp[:, r * 32:(r + 1) * 32], pT[:, r::RP],
                    ident_bf[0:16, 0:32], start=True, stop=True,
                )
            p_nat = sbw.tile([128, RP * 32], BF16, tag="p_nat", name=f"pn{ei}", bufs=2)
            nc.vector.tensor_copy(out=p_nat, in_=tp)
            return p_nat

        def compute_z(ei, p_nat):
            """col-tiled z partials: 4 r-rows concurrent, accumulated over the
            whole batch into 4 PSUM partition groups."""
            b, e = divmod(ei, E)
            xi = xi_tiles[ei // 2]
            if e == E - 1:
                xt_tiles.pop(ei // 2)
                xi_tiles.pop(ei // 2)
            if e == 0:
                zp_tiles[b] = psZP.tile([128, C], F32, tag="zp", name=f"zp{b}")
            zp = zp_tiles[b]
            for r in range(RP):
                for cc in range(2):
                    nc.tensor.matmul(
                        zp[32 * r:32 * r + 32, cc * 512:(cc + 1) * 512],
                        p_nat[:, r * 32:r * 32 + 32],
                        xi[:, ei % 2, r, cc * 512:(cc + 1) * 512],
                        start=(e == 0),
                        stop=(e == E - 1),
                        tile_position=(0, 32 * r),
                        skip_group_check=True,
                    )

        def epilogue_pre(b):
            """softmax denominator + z partials to SBUF (vector); frees zp."""
            zp = zp_tiles.pop(b)
            lp = l_parts[b]
            while len(lp) > 1:
                nxt = []
                for i in range(0, len(lp) - 1, 2):
                    ls = smalls.tile([16, 1], F32, tag="lq", name=f"ls{b}_{len(lp)}_{i}")
                    nc.vector.tensor_tensor(out=ls, in0=lp[i], in1=lp[i + 1], op=ALU.add)
                    nxt.append(ls)
                if len(lp) % 2:
                    nxt.append(lp[-1])
                lp = nxt
            linv = smalls.tile([16, 1], F32, tag="lq", name=f"li_{b}")
            nc.vector.reciprocal(out=linv, in_=lp[0])
            linvs[b] = linv
            zp_sb = sbw.tile([128, C], BF16, tag="zp_sb", name=f"zpsb{b}", bufs=2)
            nc.vector.tensor_copy(out=zp_sb, in_=zp)
            zpsb_tiles[b] = zp_sb

        def epilogue_main(b):
            """tail for batch b: reduce+normalize z, then the two projections."""
            z_sb = sbw.tile([16, C], BF16, tag="z_sb", name=f"zsb{b}", bufs=2)
            for cc in range(2):
                zred = psE.tile([16, 512], F32, tag="acc", name=f"zred{b}_{cc}")
                nc.tensor.matmul(
                    zred, sum4_state["t"],
                    zpsb_tiles[b][:, cc * 512:(cc + 1) * 512],
                    start=True, stop=True,
                )
                nc.vector.tensor_scalar_mul(
                    z_sb[:, cc * 512:(cc + 1) * 512], zred, linvs[b])

            # zT[c_p, k*16+h] for the Wv cross product
            tpz = psT.tile([128, 128], BF16, tag="tp", name=f"tpz{b}")
            for k in range(CB):
                nc.tensor.transpose(
                    tpz[:, k * 16:(k + 1) * 16],
                    z_sb[:, k * 128:(k + 1) * 128],
                    ident_bf[0:16, 0:16],
                )
            zT_sb = sbw.tile([128, 128], BF16, tag="zT", name=f"zT{b}", bufs=2)
            nc.vector.tensor_copy(out=zT_sb, in_=tpz)

            # out' = z @ Wv.T (full HxC cross)
            outp_sb = sbw.tile([16, C], BF16, tag="outp", name=f"osb{b}", bufs=2)
            for cc in range(2):
                outp = psE.tile([16, 512], F32, tag="acc", name=f"outp{b}_{cc}")
                for k in range(CB):
                    nc.tensor.matmul(
                        outp,
                        zT_sb[:, k * 16:(k + 1) * 16],
                        wts["v"][:, k, cc * 512:(cc + 1) * 512],
                        start=(k == 0), stop=(k == CB - 1),
                    )
                nc.vector.tensor_copy(
                    out=outp_sb[:, cc * 512:(cc + 1) * 512], in_=outp)

            # block-diagonal extract: out[j*128+row] lives at head 2j+(row>=64)
            tpo = psT.tile([128, 128], BF16, tag="tp", name=f"tpo{b}")
            for j in range(CB):
                nc.tensor.transpose(
                    tpo[:, j * 16:(j + 1) * 16],
                    outp_sb[:, j * 128:(j + 1) * 128],
                    ident_bf[0:16, 0:16],
                )
            oc_sb = sbw.tile([128, CB], BF16, tag="oc", name=f"oc{b}", bufs=2)
            nc.vector.tensor_copy(out=oc_sb[0:64, :], in_=tpo[0:64, 0::18])
            nc.vector.tensor_copy(out=oc_sb[64:128, :], in_=tpo[64:128, 1::18])

            # y = out @ Wp.T + bp
            y_sb = sbw.tile([1, C], F32, tag="y", name=f"y{b}", bufs=2)
            for cc in range(2):
                y_ps = psE.tile([1, 512], F32, tag="acc", name=f"yps{b}_{cc}")
                for j in range(CB):
                    nc.tensor.matmul(
                        y_ps,
                        oc_sb[:, j:j + 1],
                        wts["p"][:, j, cc * 512:(cc + 1) * 512],
                        start=(j == 0), stop=(j == CB - 1),
                    )
                nc.vector.tensor_tensor(
                    out=y_sb[:, cc * 512:(cc + 1) * 512], in0=y_ps,
                    in1=bp_state["bp"][0:1, cc * 512:(cc + 1) * 512], op=ALU.add)
            nc.sync.dma_start(out=y_d[b, :], in_=y_sb)

        # ---- schedule: two-eighth software pipeline skew; xt leads xi in
        # the DMA FIFO, weights slot in mid-stream (needed first by epi(0)).
        qh = perb.tile([128, CB, 2 * H], BF16, tag="qh", name="qh0")
        nc.sync.dma_start(out=qh, in_=qh_d[0])
        qhs[0] = qh
        sum4 = singles.tile([128, H], BF16, name="sum4")
        nc.sync.dma_start(out=sum4, in_=sum4_d[:])
        sum4_state["t"] = sum4
        emit_xt_h(0)
        mrow = perb.tile([1, N], BF16, tag="mrow", name="mrow0")
        nc.sync.dma_start(out=mrow, in_=mrow_d[0])
        mrows[0] = mrow
        emit_xt_h(1)
        emit_xi_h(0)
        emit_xt_h(2)
        emit_xi_h(1)
        bp_row = singles.tile([2, C], F32, name="bp_row")
        nc.sync.dma_start(out=bp_row, in_=_bc(bp_d[:], BPC))
        bp_state["bp"] = bp_row
        emit_small(1)
        emit_xt_h(3)

        compute_s(0)
        compute_s(1)
        for ei in range(NE):
            if ei + 4 < NE:
                emit_xt_h(ei + 4)
            if ei + 2 < NE:
                emit_xi_h(ei + 2)
            if ei == 2:
                load_w("v")
            elif ei == 4:
                load_w("p")
            p_nat = compute_tz(ei)
            if ei + 2 < NE:
                compute_s(ei + 2)
            compute_z(ei, p_nat)
            if ei % E == E - 1:
                epilogue_pre(ei // E)
            if ei == E:
                epilogue_main(0)
        epilogue_main(1)

    nc.compile()
    return nc


def _ensure_ntff_hook():
    """The agent image's antenv lacks axon_hooks; synthesize it and install
    the ctypes NTFF profile hook from trn_boot so trace=True works."""
    import sys
    import types
    try:
        from antenv.axon_hooks import get_axon_ntff_profile_hook  # noqa: F401
        return
    except ImportError:
        pass
    import antenv
    mod = types.ModuleType("antenv.axon_hooks")
    state = {}
    mod.set_axon_ntff_profile_hook = lambda h: state.__setitem__("h", h)
    mod.get_axon_ntff_profile_hook = lambda: state.get("h")
    sys.modules["antenv.axon_hooks"] = mod
    antenv.axon_hooks = mod
    try:
        from trn_agent_boot.trn_boot import _ntff_profile_via_ctypes
        mod.set_axon_ntff_profile_hook(
            _ntff_profile_via_ctypes("/opt/axon/libaxon_pjrt.so")
        )
    except Exception:
        pass


_NC_CACHE = None


def _get_module():
    global _NC_CACHE
    if _NC_CACHE is None:
        _NC_CACHE = build_module()
    return _NC_CACHE


def _prep_inputs(inputs):
    """Host-side prep: bf16/fp8 casts, DMA-friendly reorders, per-batch qhat."""
    import ml_dtypes
    bf16 = ml_dtypes.bfloat16
    f8 = ml_dtypes.float8_e4m3

    x = np.ascontiguousarray(inputs["x"], dtype=np.float32)       # (B,N,C)
    mask = np.ascontiguousarray(inputs["mask"], dtype=np.float32)
    Wq = np.asarray(inputs["Wq"], dtype=np.float32)
    Wk = np.asarray(inputs["Wk"], dtype=np.float32)

    xb = x.astype(bf16)                                            # (B,N,C)
    # transposed copy in fp8, reordered to [B, Q, 128, CB, QN]:
    # (b,q,p,k,n') = x[b, q*QN+n', k*128+p]
    xt = x.transpose(0, 2, 1)                                      # (B,C,N)
    xt8 = np.ascontiguousarray(
        xt.reshape(B, CB, 128, Q, QN).transpose(0, 3, 2, 1, 4)
    ).astype(f8)

    # qhat[b,h,:] = sum_d (x[b,0] @ Wq.T * scale)[h*64+d] * Wk[h*64+d,:]
    q = (x[:, 0, :].astype(np.float64) @ Wq.T.astype(np.float64)) * SCALE  # (B,C)
    qhd = q.reshape(B, H, D)
    Wkh = Wk.reshape(H, D, C).astype(np.float64)
    qhat = np.einsum("bhd,hdc->bhc", qhd, Wkh)                     # (B,H,C)
    # [B, 128, CB, 2H]: (b,p,k,h) = qhat[b, h, k*128+p], heads 16-31 zero-padded
    qhT = np.ascontiguousarray(
        qhat.transpose(0, 2, 1).reshape(B, CB, 128, H).transpose(0, 2, 1, 3))
    qhT = np.concatenate([qhT, np.zeros_like(qhT)], axis=3).astype(bf16)

    # mask_full = [0, mask[b]] as a single bf16 row per batch
    mrow = np.concatenate(
        [np.zeros((B, 1), np.float32), mask], axis=1).astype(bf16)  # (B,N)

    def reorder_w(w):  # (C,C) -> [128, CB, C] with (p,k,c) = W[c, k*128+p]
        wt = np.ascontiguousarray(np.asarray(w, np.float32).T)      # (C,C) W.T
        return np.ascontiguousarray(
            wt.reshape(CB, 128, C).transpose(1, 0, 2)).astype(bf16)

    # reduction stationary for the 4 col-tiled partition groups:
    # sum4[32g+h, h] = 1  (g = array col-group, h = head)
    sum4 = np.zeros((128, H), dtype=np.float32)
    for g in range(4):
        sum4[32 * g:32 * g + H, :] = np.eye(H)
    shared = {
        "WvT": reorder_w(inputs["Wv"]),
        "WpT": reorder_w(inputs["Wp"]),
        "bp": np.ascontiguousarray(inputs["bp"], dtype=np.float32),
        "sum4": sum4.astype(bf16),
    }
    in_maps = []
    for c in range(NCORES):
        sl = slice(c * BPC, (c + 1) * BPC)
        m = {
            "xb": xb[sl], "xt8": xt8[sl], "qhT": qhT[sl],
            "mrow": mrow[sl],
        }
        m.update(shared)
        in_maps.append(m)
    return in_maps


def run(inputs, trace=False):
    if trace:
        _ensure_ntff_hook()
    nc = _get_module()
    in_maps = _prep_inputs(inputs)
    res = bass_utils.run_bass_kernel_spmd(
        nc, in_maps, core_ids=list(range(NCORES)), trace=trace
    )
    ys = [res.results[c]["y"] for c in range(NCORES)]
    out = np.concatenate(ys, axis=0).reshape(B, 1, C)
    return out, res


def kernel(**inputs):
    out, _ = run(inputs, trace=False)
    return out


if __name__ == "__main__":
    rng = np.random.default_rng(0)
    ins = {
        "x": rng.standard_normal((B, N, C), dtype=np.float32),
        "mask": np.zeros((B, N - 1), dtype=np.float32),
        "Wq": (rng.standard_normal((C, C)) * 0.02).astype(np.float32),
        "Wk": (rng.standard_normal((C, C)) * 0.02).astype(np.float32),
        "Wv": (rng.standard_normal((C, C)) * 0.02).astype(np.float32),
        "Wp": (rng.standard_normal((C, C)) * 0.02).astype(np.float32),
        "bp": np.zeros((C,), dtype=np.float32),
    }
    y = kernel(**ins)
    print(y.shape, y.dtype, np.abs(y).mean())


# revision 27
# speedup vs baseline: 1.1855x; 1.0204x over previous
"""Trainium2 Bass kernel for single-CLS-query attention.

Reference computation (per batch b):
    q   = (x[b,0,:] @ Wq.T) * d**-0.5                  # (C,)  single CLS query
    k   = x[b] @ Wk.T ; v = x[b] @ Wv.T                # (N,C)
    s   = per-head dot(q, k) + mask                    # (N,H)
    p   = softmax(s, axis=N)
    out = per-head sum_n p[n,h] v[n,h*64:(h+1)*64]     # (C,)
    y   = out @ Wp.T + bp

Key algebraic restructuring (exploits the single query):
    qhat[h,:] = sum_d q[h*64+d] * Wk[h*64+d,:]         # (H,C)  fold q through Wk
    s         = x @ qhat.T                             # skinny matmul, no k!
    z[h,:]    = sum_n p[n,h] * x[b,n,:]                # (H,C)  fold p into x
    out'      = z @ Wv.T  (full 16x1024 cross)         # block-diag extract -> out
This removes both dense projections x@Wk.T / x@Wv.T (~137 GFLOP -> ~2 GFLOP)
and makes the kernel memory-bound on streaming x.

x is streamed twice: once transposed (C on partitions) for the s-matmul, once
natural (N on partitions) for the z accumulation. The transposed copy only
feeds the softmax logits, so it ships as fp8e4m3 (half the bytes; measured
end-to-end rel-err ~9e-3 vs the 2e-2 gate). The natural copy stays bf16.
Both copies are host-reordered so every DMA lands as large fully-contiguous
per-partition descriptors (8-16KB), one dma_start per quarter-batch; the
profiled baseline lost ~90us to per-dma_start sync-engine serialization
(163 issues x ~0.7us) plus repeated HAM clock-throttle from TensorE gaps.

The additive mask is folded into the s-matmul PSUM group as a 9th
accumulation matmul (ones[1,16].T @ mask_row[1,n]), and the softmax
denominator comes free from the Exp activation's accum_out, so the whole
p-production path is: matmuls -> one fused exp -> 8 tiny transposes.

Sharding: data-parallel over batch. 8 cores x 2 batches each. No collectives.
softmax is computed without max-subtraction: logits here are ~N(0, 0.4), far
inside fp32 exp range (mask is additive zeros in this problem's distribution).
"""

import numpy as np
from contextlib import ExitStack

import concourse.bass as bass
from concourse import bacc
import concourse.tile as tile
from concourse import mybir
from concourse import bass_utils
from concourse.masks import make_identity

B, N, C, H, D = 16, 4096, 1024, 16, 64
NCORES = 8
BPC = B // NCORES          # batches per core
SCALE = float(D) ** -0.5
F32 = mybir.dt.float32
BF16 = mybir.dt.bfloat16
FP8 = mybir.dt.float8e4
CB = C // 128              # 8 contraction blocks of 128 channels
Q = 4                      # quarters per batch (DMA granule)
QN = N // Q                # 1024 rows per quarter
E = 8                      # eighths per batch (PSUM/pipeline granule)
EN = N // E                # 512 rows per eighth
RP = EN // 128             # 4 rows per partition within an eighth

AF = mybir.ActivationFunctionType
ALU = mybir.AluOpType


def _bc(ap_slice, parts):
    """Broadcast an AP (leading dim of size 1, or 1-D) over `parts` partitions."""
    dims = [list(p) for p in ap_slice.ap]
    if len(dims) > 1 and dims[0][1] == 1:
        dims = dims[1:]
    return bass.AP(
        tensor=ap_slice.tensor,
        offset=ap_slice.offset,
        ap=[[0, parts]] + dims,
    )


def build_module():
    nc = bacc.Bacc(target_bir_lowering=False, trn_type="TRN2")

    x_d = nc.dram_tensor("xb", [BPC, N, C], BF16, kind="ExternalInput")
    xt8_d = nc.dram_tensor("xt8", [BPC, Q, 128, CB, QN], FP8, kind="ExternalInput")
    qh_d = nc.dram_tensor("qhT", [BPC, 128, CB, 2 * H], BF16, kind="ExternalInput")
    mrow_d = nc.dram_tensor("mrow", [BPC, N], BF16, kind="ExternalInput")
    sum4_d = nc.dram_tensor("sum4", [128, H], BF16, kind="ExternalInput")
    wv_d = nc.dram_tensor("WvT", [128, CB, C], BF16, kind="ExternalInput")
    wp_d = nc.dram_tensor("WpT", [128, CB, C], BF16, kind="ExternalInput")
    bp_d = nc.dram_tensor("bp", [C], F32, kind="ExternalInput")
    y_d = nc.dram_tensor("y", [BPC, C], F32, kind="ExternalOutput")

    with tile.TileContext(nc) as tc, ExitStack() as ctx:
        singles = ctx.enter_context(tc.tile_pool(name="singles", bufs=1))
        perb = ctx.enter_context(tc.tile_pool(name="perb", bufs=2))
        xtq = ctx.enter_context(tc.tile_pool(name="xtq", bufs=5))
        xinq = ctx.enter_context(tc.tile_pool(name="xinq", bufs=5))
        sbw = ctx.enter_context(tc.tile_pool(name="sbw", bufs=3))
        smalls = ctx.enter_context(tc.tile_pool(name="smalls", bufs=12))
        psST = ctx.enter_context(tc.tile_pool(name="psST", bufs=2, space="PSUM"))
        psZP = ctx.enter_context(tc.tile_pool(name="psZP", bufs=1, space="PSUM"))
        psE = ctx.enter_context(tc.tile_pool(name="psE", bufs=2, space="PSUM"))
        psT = ctx.enter_context(tc.tile_pool(name="psT", bufs=2, space="PSUM"))

        ident_bf = singles.tile([128, 128], BF16)
        make_identity(nc, ident_bf)

        ones16 = singles.tile([1, H], BF16)
        nc.vector.memset(ones16, 1.0)

        # per-batch tiny tensors: folded query (C,H) and mask row (1,N)
        qhs, mrows = [None] * BPC, [None] * BPC
        bp_state = {}

        def emit_small(b):
            qh = perb.tile([128, CB, 2 * H], BF16, tag="qh", name=f"qh{b}")
            nc.sync.dma_start(out=qh, in_=qh_d[b])
            qhs[b] = qh
            mrow = perb.tile([1, N], BF16, tag="mrow", name=f"mrow{b}")
            nc.sync.dma_start(out=mrow, in_=mrow_d[b])
            mrows[b] = mrow

        # streamed quarter tiles: transposed fp8 (s input) + natural bf16 (z input)
        qtiles = {}

        xt_tiles, xi_tiles = {}, {}

        # stream DMAs land at eighth granularity: the per-eighth wait when
        # compute catches the (saturated) DMA stays under the ~3.4us HAM
        # window, so the PE clock never drops to 1.2 GHz mid-stream.
        def emit_xt_h(ei):
            # one DMA per quarter (1KB descriptors; a per-eighth split was
            # measured slower: 512B descriptors drop DMA efficiency ~15%)
            qi, h = divmod(ei, 2)
            if h != 0:
                return
            b, q = divmod(qi, Q)
            xt_tiles[qi] = xtq.tile(
                [128, CB, QN], FP8, tag="xt", name=f"xt{b}_{q}")
            nc.sync.dma_start(out=xt_tiles[qi], in_=xt8_d[b, q])

        def emit_xi_h(ei):
            qi, h = divmod(ei, 2)
            if h != 0:
                return
            b, q = divmod(qi, Q)
            xi = xinq.tile([128, 2, RP, C], BF16, tag="xin", name=f"xi{b}_{q}")
            src = x_d[b, q * QN:(q + 1) * QN, :].rearrange(
                "(e p r) c -> p e r c", e=2, r=RP)
            nc.sync.dma_start(out=xi, in_=src)
            xi_tiles[qi] = xi

        wts = {}

        def load_w(nm):
            wt_d = {"v": wv_d, "p": wp_d}[nm]
            w = singles.tile([128, CB, C], BF16, tag=f"w_{nm}", name=f"w_{nm}")
            nc.sync.dma_start(out=w, in_=wt_d[:])
            wts[nm] = w

        NE = BPC * E           # 16 eighths total
        spsb_tiles = {}
        sT_tiles = {}
        pT_tiles = {}
        zp_tiles = {}
        l_parts = {b: [] for b in range(BPC)}
        linvs = {}
        zpsb_tiles = {}
        sum4_state = {}

        def compute_s(ei):
            """s-matmuls for eighth ei straight into one PSUM bank, then exp.
            (A col-tiled variant was tried here: the partials->SBUF->reduce
            hops starved TensorE and lost more to HAM throttling than the
            concurrency won back.)"""
            b, e = divmod(ei, E)
            xt = xt_tiles[ei // 2]
            cols = slice((ei % 2) * EN, (ei % 2 + 1) * EN)
            sT = psST.tile([16, EN], F32, tag="sT", name=f"sT{ei}")
            for k in range(CB):
                nc.tensor.matmul(
                    sT, qhs[b][:, k, 0:H], xt[:, k, cols],
                    start=(k == 0), stop=False,
                )
            nc.tensor.matmul(
                sT, ones16, mrows[b][:, e * EN:(e + 1) * EN],
                start=False, stop=True,
            )
            # fused: PSUM->SBUF move + exp + softmax-denominator partial
            pT = sbw.tile([16, EN], BF16, tag="pT", name=f"pT{ei}", bufs=3)
            lq = smalls.tile([16, 1], F32, tag="lq", name=f"lq{ei}")
            nc.scalar.activation(out=pT, in_=sT, func=AF.Exp, accum_out=lq)
            l_parts[b].append(lq)
            pT_tiles[ei] = pT

        def compute_tz(ei):
            """transpose p to natural layout (n on partitions)."""
            pT = pT_tiles.pop(ei)
            tp = psT.tile([128, RP * 32], F32, tag="tp", name=f"tp{ei}")
            for r in range(RP):
                # plain matmul against a zero-padded identity: cols 16-31 of
                # each 32-group come out as hard zeros (stationary padding for
                # the col-tiled z matmuls)
                nc.tensor.matmul(
                    tp[:, r * 32:(r + 1) * 32], pT[:, r::RP],
                    ident_bf[0:16, 0:32], start=True, stop=True,
                )
            p_nat = sbw.tile([128, RP * 32], BF16, tag="p_nat", name=f"pn{ei}", bufs=2)
            nc.vector.tensor_copy(out=p_nat, in_=tp)
            return p_nat

        def compute_z(ei, p_nat):
            """col-tiled z partials: 4 r-rows concurrent, accumulated over the
            whole batch into 4 PSUM partition groups."""
            b, e = divmod(ei, E)
            xi = xi_tiles[ei // 2]
            if e == E - 1:
                xt_tiles.pop(ei // 2)
                xi_tiles.pop(ei // 2)
            if e == 0:
                zp_tiles[b] = psZP.tile([128, C], F32, tag="zp", name=f"zp{b}")
            zp = zp_tiles[b]
            for r in range(RP):
                for cc in range(2):
                    nc.tensor.matmul(
                        zp[32 * r:32 * r + 32, cc * 512:(cc + 1) * 512],
                        p_nat[:, r * 32:r * 32 + 32],
                        xi[:, ei % 2, r, cc * 512:(cc + 1) * 512],
                        start=(e == 0),
                        stop=(e == E - 1),
                        tile_position=(0, 32 * r),
                        skip_group_check=True,
                    )

        def epilogue_pre(b):
            """softmax denominator + z partials to SBUF (vector); frees zp."""
            zp = zp_tiles.pop(b)
            lp = l_parts[b]
            while len(lp) > 1:
                nxt = []
                for i in range(0, len(lp) - 1, 2):
                    ls = smalls.tile([16, 1], F32, tag="lq", name=f"ls{b}_{len(lp)}_{i}")
                    nc.vector.tensor_tensor(out=ls, in0=lp[i], in1=lp[i + 1], op=ALU.add)
                    nxt.append(ls)
                if len(lp) % 2:
                    nxt.append(lp[-1])
                lp = nxt
            linv = smalls.tile([16, 1], F32, tag="lq", name=f"li_{b}")
            nc.vector.reciprocal(out=linv, in_=lp[0])
            linvs[b] = linv
            zp_sb = sbw.tile([128, C], BF16, tag="zp_sb", name=f"zpsb{b}", bufs=2)
            nc.vector.tensor_copy(out=zp_sb, in_=zp)
            zpsb_tiles[b] = zp_sb

        def epilogue_main(b):
            """tail for batch b: reduce+normalize z, then the two projections."""
            z_sb = sbw.tile([16, C], BF16, tag="z_sb", name=f"zsb{b}", bufs=2)
            for cc in range(2):
                zred = psE.tile([16, 512], F32, tag="acc", name=f"zred{b}_{cc}")
                nc.tensor.matmul(
                    zred, sum4_state["t"],
                    zpsb_tiles[b][:, cc * 512:(cc + 1) * 512],
                    start=True, stop=True,
                )
                nc.vector.tensor_scalar_mul(
                    z_sb[:, cc * 512:(cc + 1) * 512], zred, linvs[b])

            # zT[c_p, k*16+h] for the Wv cross product
            tpz = psT.tile([128, 128], BF16, tag="tp", name=f"tpz{b}")
            for k in range(CB):
                nc.tensor.transpose(
                    tpz[:, k * 16:(k + 1) * 16],
                    z_sb[:, k * 128:(k + 1) * 128],
                    ident_bf[0:16, 0:16],
                )
            zT_sb = sbw.tile([128, 128], BF16, tag="zT", name=f"zT{b}", bufs=2)
            nc.vector.tensor_copy(out=zT_sb, in_=tpz)

            # out' = z @ Wv.T (full HxC cross)
            outp_sb = sbw.tile([16, C], BF16, tag="outp", name=f"osb{b}", bufs=2)
            for cc in range(2):
                outp = psE.tile([16, 512], F32, tag="acc", name=f"outp{b}_{cc}")
                for k in range(CB):
                    nc.tensor.matmul(
                        outp,
                        zT_sb[:, k * 16:(k + 1) * 16],
                        wts["v"][:, k, cc * 512:(cc + 1) * 512],
                        start=(k == 0), stop=(k == CB - 1),
                    )
                nc.vector.tensor_copy(
                    out=outp_sb[:, cc * 512:(cc + 1) * 512], in_=outp)

            # block-diagonal extract: out[j*128+row] lives at head 2j+(row>=64)
            tpo = psT.tile([128, 128], BF16, tag="tp", name=f"tpo{b}")
            for j in range(CB):
                nc.tensor.transpose(
                    tpo[:, j * 16:(j + 1) * 16],
                    outp_sb[:, j * 128:(j + 1) * 128],
                    ident_bf[0:16, 0:16],
                )
            oc_sb = sbw.tile([128, CB], BF16, tag="oc", name=f"oc{b}", bufs=2)
            nc.vector.tensor_copy(out=oc_sb[0:64, :], in_=tpo[0:64, 0::18])
            nc.vector.tensor_copy(out=oc_sb[64:128, :], in_=tpo[64:128, 1::18])

            # y = out @ Wp.T + bp
            y_sb = sbw.tile([1, C], F32, tag="y", name=f"y{b}", bufs=2)
            for cc in range(2):
                y_ps = psE.tile([1, 512], F32, tag="acc", name=f"yps{b}_{cc}")
                for j in range(CB):
                    nc.tensor.matmul(
                        y_ps,
                        oc_sb[:, j:j + 1],
                        wts["p"][:, j, cc * 512:(cc + 1) * 512],
                        start=(j == 0), stop=(j == CB - 1),
                    )
                nc.vector.tensor_tensor(
                    out=y_sb[:, cc * 512:(cc + 1) * 512], in0=y_ps,
                    in1=bp_state["bp"][0:1, cc * 512:(cc + 1) * 512], op=ALU.add)
            nc.sync.dma_start(out=y_d[b, :], in_=y_sb)

        # ---- schedule: two-eighth software pipeline skew; xt leads xi in
        # the DMA FIFO, weights slot in mid-stream (needed first by epi(0)).
        emit_xt_h(0)
        qh = perb.tile([128, CB, 2 * H], BF16, tag="qh", name="qh0")
        nc.sync.dma_start(out=qh, in_=qh_d[0])
        qhs[0] = qh
        sum4 = singles.tile([128, H], BF16, name="sum4")
        nc.sync.dma_start(out=sum4, in_=sum4_d[:])
        sum4_state["t"] = sum4
        mrow = perb.tile([1, N], BF16, tag="mrow", name="mrow0")
        nc.sync.dma_start(out=mrow, in_=mrow_d[0])
        mrows[0] = mrow
        emit_xi_h(0)
        emit_xt_h(2)
        bp_row = singles.tile([2, C], F32, name="bp_row")
        nc.sync.dma_start(out=bp_row, in_=_bc(bp_d[:], BPC))
        bp_state["bp"] = bp_row
        emit_small(1)
        emit_xt_h(3)

        compute_s(0)
        compute_s(1)
        for ei in range(NE):
            if ei + 4 < NE:
                emit_xt_h(ei + 4)
            if ei + 2 < NE:
                emit_xi_h(ei + 2)
            if ei == 2:
                load_w("v")
            elif ei == 4:
                load_w("p")
            p_nat = compute_tz(ei)
            if ei + 2 < NE:
                compute_s(ei + 2)
            compute_z(ei, p_nat)
            if ei % E == E - 1:
                epilogue_pre(ei // E)
            if ei == E:
                epilogue_main(0)
        epilogue_main(1)

    nc.compile()
    return nc


def _ensure_ntff_hook():
    """The agent image's antenv lacks axon_hooks; synthesize it and install
    the ctypes NTFF profile hook from trn_boot so trace=True works."""
    import sys
    import types
    try:
        from antenv.axon_hooks import get_axon_ntff_profile_hook  # noqa: F401
        return
    except ImportError:
        pass
    import antenv
    mod = types.ModuleType("antenv.axon_hooks")
    state = {}
    mod.set_axon_ntff_profile_hook = lambda h: state.__setitem__("h", h)
    mod.get_axon_ntff_profile_hook = lambda: state.get("h")
    sys.modules["antenv.axon_hooks"] = mod
    antenv.axon_hooks = mod
    try:
        from trn_agent_boot.trn_boot import _ntff_profile_via_ctypes
        mod.set_axon_ntff_profile_hook(
            _ntff_profile_via_ctypes("/opt/axon/libaxon_pjrt.so")
        )
    except Exception:
        pass


_NC_CACHE = None


def _get_module():
    global _NC_CACHE
    if _NC_CACHE is None:
        _NC_CACHE = build_module()
    return _NC_CACHE


def _prep_inputs(inputs):
    """Host-side prep: bf16/fp8 casts, DMA-friendly reorders, per-batch qhat."""
    import ml_dtypes
    bf16 = ml_dtypes.bfloat16
    f8 = ml_dtypes.float8_e4m3

    x = np.ascontiguousarray(inputs["x"], dtype=np.float32)       # (B,N,C)
    mask = np.ascontiguousarray(inputs["mask"], dtype=np.float32)
    Wq = np.asarray(inputs["Wq"], dtype=np.float32)
    Wk = np.asarray(inputs["Wk"], dtype=np.float32)

    xb = x.astype(bf16)                                            # (B,N,C)
    # transposed copy in fp8, reordered to [B, Q, 128, CB, QN]:
    # (b,q,p,k,n') = x[b, q*QN+n', k*128+p]
    xt = x.transpose(0, 2, 1)                                      # (B,C,N)
    xt8 = np.ascontiguousarray(
        xt.reshape(B, CB, 128, Q, QN).transpose(0, 3, 2, 1, 4)
    ).astype(f8)

    # qhat[b,h,:] = sum_d (x[b,0] @ Wq.T * scale)[h*64+d] * Wk[h*64+d,:]
    q = (x[:, 0, :].astype(np.float64) @ Wq.T.astype(np.float64)) * SCALE  # (B,C)
    qhd = q.reshape(B, H, D)
    Wkh = Wk.reshape(H, D, C).astype(np.float64)
    qhat = np.einsum("bhd,hdc->bhc", qhd, Wkh)                     # (B,H,C)
    # [B, 128, CB, 2H]: (b,p,k,h) = qhat[b, h, k*128+p], heads 16-31 zero-padded
    qhT = np.ascontiguousarray(
        qhat.transpose(0, 2, 1).reshape(B, CB, 128, H).transpose(0, 2, 1, 3))
    qhT = np.concatenate([qhT, np.zeros_like(qhT)], axis=3).astype(bf16)

    # mask_full = [0, mask[b]] as a single bf16 row per batch
    mrow = np.concatenate(
        [np.zeros((B, 1), np.float32), mask], axis=1).astype(bf16)  # (B,N)

    def reorder_w(w):  # (C,C) -> [128, CB, C] with (p,k,c) = W[c, k*128+p]
        wt = np.ascontiguousarray(np.asarray(w, np.float32).T)      # (C,C) W.T
        return np.ascontiguousarray(
            wt.reshape(CB, 128, C).transpose(1, 0, 2)).astype(bf16)

    # reduction stationary for the 4 col-tiled partition groups:
    # sum4[32g+h, h] = 1  (g = array col-group, h = head)
    sum4 = np.zeros((128, H), dtype=np.float32)
    for g in range(4):
        sum4[32 * g:32 * g + H, :] = np.eye(H)
    shared = {
        "WvT": reorder_w(inputs["Wv"]),
        "WpT": reorder_w(inputs["Wp"]),
        "bp": np.ascontiguousarray(inputs["bp"], dtype=np.float32),
        "sum4": sum4.astype(bf16),
    }
    in_maps = []
    for c in range(NCORES):
        sl = slice(c * BPC, (c + 1) * BPC)
        m = {
            "xb": xb[sl], "xt8": xt8[sl], "qhT": qhT[sl],
            "mrow": mrow[sl],
        }
        m.update(shared)
        in_maps.append(m)
    return in_maps


def run(inputs, trace=False):
    if trace:
        _ensure_ntff_hook()
    nc = _get_module()
    in_maps = _prep_inputs(inputs)
    res = bass_utils.run_bass_kernel_spmd(
        nc, in_maps, core_ids=list(range(NCORES)), trace=trace
    )
    ys = [res.results[c]["y"] for c in range(NCORES)]
    out = np.concatenate(ys, axis=0).reshape(B, 1, C)
    return out, res


def kernel(**inputs):
    out, _ = run(inputs, trace=False)
    return out


if __name__ == "__main__":
    rng = np.random.default_rng(0)
    ins = {
        "x": rng.standard_normal((B, N, C), dtype=np.float32),
        "mask": np.zeros((B, N - 1), dtype=np.float32),
        "Wq": (rng.standard_normal((C, C)) * 0.02).astype(np.float32),
        "Wk": (rng.standard_normal((C, C)) * 0.02).astype(np.float32),
        "Wv": (rng.standard_normal((C, C)) * 0.02).astype(np.float32),
        "Wp": (rng.standard_normal((C, C)) * 0.02).astype(np.float32),
        "bp": np.zeros((C,), dtype=np.float32),
    }
    y = kernel(**ins)
    print(y.shape, y.dtype, np.abs(y).mean())
